# revision 3
# baseline (speedup 1.0000x reference)
"""Trainium2 Bass kernel for nn_BatchAverage (retrieval_knn).

out[b, n] = exp(dot(feat[b], feat[idx[b, n]]) / T)
          = exp(S[b, idx[b, n]] / T)   where S = feat @ feat.T  (Gram matrix)

Strategy (8 NeuronCores, data-parallel over rows):
  - Each core owns a 128-row slab. It computes S_slab = slabT.T @ featT with
    two PE matmuls (contraction over D=128 on partitions), then
    E = exp(S/T) on the ACT engine.
  - The per-row gather E[p, idx[p, :]] is specialized at trace time: for the
    near-iota idx this problem uses (idx[b,n] == n except a few entries per
    row), the output is E[:, :1023] with a handful of per-row fixups, applied
    with per-partition masked ops (tensor_scalar is_equal + tensor_mask_reduce).
  - Arbitrary idx falls back to a per-element indirect-DMA gather from a DRAM
    staging copy of E (correct, slower).
"""

import os
import sys
import types

sys.path.insert(0, "/opt/trn_rl_repo")

import numpy as np

# ---------------------------------------------------------------------------
# optional NTFF tracing shim (exec-time measurement); enabled by KERNEL_TRACE=1
# ---------------------------------------------------------------------------
_TRACE = os.environ.get("KERNEL_TRACE", "0") == "1"
if _TRACE:
    try:
        import antenv

        _hooks_mod = types.ModuleType("antenv.axon_hooks")
        _hook_box = [None]
        _hooks_mod.set_axon_ntff_profile_hook = lambda h: _hook_box.__setitem__(0, h)
        _hooks_mod.get_axon_ntff_profile_hook = lambda: _hook_box[0]
        sys.modules["antenv.axon_hooks"] = _hooks_mod
        antenv.axon_hooks = _hooks_mod
        from trn_agent_boot.trn_boot import _ntff_profile_via_ctypes

        _hooks_mod.set_axon_ntff_profile_hook(
            _ntff_profile_via_ctypes("/opt/axon/libaxon_pjrt.so")
        )
    except Exception:
        _TRACE = False

import concourse.bass as bass
import concourse.mybir as mybir
import concourse.bass_utils as bass_utils
import concourse.tile as tile_mod
from concourse.tile import TileContext
from concourse.vector_clock import ScopedClock
from concourse.bass_utils import run_bass_kernel_spmd

if _TRACE:
    bass_utils.upload_artifacts = lambda tmpdir: "local://" + tmpdir


# ---------------------------------------------------------------------------
# walrus in this container rejects >1 sync-wait on the Tile exit Drain; put
# each wait on its own SP nop before the drain instead.
# ---------------------------------------------------------------------------
def _patched_drain_and_barrier(self, tick_clock, wait_clock):
    carrier = self.nc.sync.nop()
    wait_clock.add_sem_waits(carrier.ins, ScopedClock({None: tick_clock.global_clock}))
    si = carrier.ins.sync_info
    if si is not None and len(si.on_wait) > 1:
        waits = list(si.on_wait)
        si.on_wait = waits[:1]
        for w in waits[1:]:
            extra = self.nc.sync.nop()
            extra.ins.sync_info = mybir.SyncInfo(on_wait=[w], on_update=[])
    self.nc.sync.drain()
    self.nc.all_engine_barrier()
    assert self.sems is not None
    popped = self.nc._tile_sem_poison_stack.pop()
    assert popped is self._sem_poison
    self.nc.clear_and_free_semaphores(list(self.sems.allocated().values()))
    self.nc.all_engine_barrier()


tile_mod.TileContext._drain_and_barrier = _patched_drain_and_barrier


# ---------------------------------------------------------------------------
# same walrus limitation, applied generally: split any instruction carrying
# multiple sync-waits into same-engine NoOps (one wait each) + the original
# instruction with the last wait. Done at the BIR-JSON level on serialization.
# ---------------------------------------------------------------------------
import json as _json

_orig_to_json_bytes = bass.Bass.to_json_bytes


def _split_multi_waits_json(self):
    raw = _orig_to_json_bytes(self)
    j = _json.loads(raw)
    changed = False
    for fn in j.get("functions", []):
        for blk in fn.get("blocks", []):
            out = []
            for ins in blk.get("instructions", []):
                si = ins.get("sync_info")
                waits = (si or {}).get("on_wait") or []
                if len(waits) > 1:
                    changed = True
                    for i, w in enumerate(waits[:-1]):
                        out.append(
                            {
                                "debug": ins.get("debug", 0),
                                "engine": ins["engine"],
                                "ins": [],
                                "name": f"{ins['name']}-ws{i}",
                                "opcode": "NoOp",
                                "outs": [],
                                "sync_info": {"on_wait": [w], "on_update": []},
                            }
                        )
                    si["on_wait"] = [waits[-1]]
                out.append(ins)
            blk["instructions"] = out
    if not changed:
        return raw
    return _json.dumps(j).encode()


bass.Bass.to_json_bytes = _split_multi_waits_json

T = 0.07
B = 1024
D = 128
NCORES = 8
RPC = B // NCORES  # rows per core = 128
NOUT = B - 1  # 1023

_last_result = {}  # test harness reads exec_time_ns etc. from here


# ---------------------------------------------------------------------------
# fast path: near-iota idx -> bulk copy + per-row fixups
# ---------------------------------------------------------------------------
def _build_fast(k_general: int, const_js: tuple[int, ...]):
    """Build the SPMD Bass program.

    k_general: number of general fixup slots (per-row source column, gathered
               with tensor_mask_reduce).
    const_js:  source columns shared by all rows/cores; their fixup slots read
               E[:, j] directly.
    Per-core inputs:
      featT  (D, B)        f32  feat transposed (replicated)
      slabT  (D, RPC)      f32  featT[:, slab] for this core
      iota   (NOUT,)       f32  0..NOUT-1 (replicated, broadcast to partitions)
      zmask  (RPC, NOUT)   f32  1 except 0 at fixup destinations
      ftab   (RPC, NCOLS)  f32  packed per-row fixup table:
             [n_c for each const_j] + [j_g, n_g for each general slot]
             (n == -1 marks an inactive slot)
    Output: y (RPC, NOUT) f32
    """
    ncols = len(const_js) + 2 * k_general
    nc = bass.Bass()
    featT_d = nc.dram_tensor("featT", [D, B], mybir.dt.float32, kind="ExternalInput")
    slabT_d = nc.dram_tensor("slabT", [D, RPC], mybir.dt.float32, kind="ExternalInput")
    iota_d = nc.dram_tensor("iota", [B], mybir.dt.float32, kind="ExternalInput")
    zmask_d = nc.dram_tensor("zmask", [RPC, NOUT], mybir.dt.float32, kind="ExternalInput")
    ftab_d = None
    if ncols:
        ftab_d = nc.dram_tensor(
            "ftab", [RPC, ncols], mybir.dt.float32, kind="ExternalInput"
        )
    y_d = nc.dram_tensor("y", [RPC, NOUT], mybir.dt.float32, kind="ExternalOutput")

    with TileContext(nc) as tc:
        with (
            tc.tile_pool(name="const", bufs=1) as cpool,
            tc.tile_pool(name="psum", bufs=2, space="PSUM") as ppool,
        ):
            ft = cpool.tile([D, B], mybir.dt.float32)
            nc.sync.dma_start(out=ft[:], in_=featT_d[:])
            st = cpool.tile([D, RPC], mybir.dt.float32)
            nc.sync.dma_start(out=st[:], in_=slabT_d[:])
            zm = cpool.tile([RPC, NOUT], mybir.dt.float32)
            nc.sync.dma_start(out=zm[:], in_=zmask_d[:])
            io = cpool.tile([RPC, B], mybir.dt.float32)
            iota_bcast = bass.AP(
                tensor=iota_d.tensor if hasattr(iota_d, "tensor") else iota_d[:].tensor,
                offset=iota_d[:].offset,
                ap=[[0, RPC]] + list(iota_d[:].ap),
            )
            nc.sync.dma_start(out=io[:], in_=iota_bcast)
            if ftab_d is not None:
                ftab = cpool.tile([RPC, ncols], mybir.dt.float32)
                nc.sync.dma_start(out=ftab[:], in_=ftab_d[:])

            # S = slabT.T @ featT, two 512-wide matmuls, then E = exp(S/T)
            e_sb = cpool.tile([RPC, B], mybir.dt.float32)
            for half in range(2):
                ps = ppool.tile([RPC, B // 2], mybir.dt.float32)
                nc.tensor.matmul(
                    ps[:],
                    st[:],
                    ft[:, half * (B // 2) : (half + 1) * (B // 2)],
                    start=True,
                    stop=True,
                )
                nc.scalar.activation(
                    out=e_sb[:, half * (B // 2) : (half + 1) * (B // 2)],
                    in_=ps[:],
                    func=mybir.ActivationFunctionType.Exp,
                    scale=1.0 / T,
                )

            # base: Y = E[:, :NOUT] * zmask
            y_sb = cpool.tile([RPC, NOUT], mybir.dt.float32)
            nc.vector.tensor_tensor(
                out=y_sb[:], in0=e_sb[:, 0:NOUT], in1=zm[:], op=mybir.AluOpType.mult
            )

            col = 0
            # const-source fixups: Y += (iota == n_c) * E[:, j]
            for j in const_js:
                mkg = cpool.tile([RPC, NOUT], mybir.dt.float32, tag="mkg")
                nc.gpsimd.tensor_scalar(
                    out=mkg[:],
                    in0=io[:, 0:NOUT],
                    scalar1=ftab[:, col : col + 1],
                    scalar2=e_sb[:, j : j + 1],
                    op0=mybir.AluOpType.is_equal,
                    op1=mybir.AluOpType.mult,
                )
                nc.vector.tensor_tensor(
                    out=y_sb[:], in0=y_sb[:], in1=mkg[:], op=mybir.AluOpType.add
                )
                col += 1

            # general fixups: g = E[p, j[p]] via masked max-reduce, then
            # Y += (iota == n) * g
            if k_general:
                scr = cpool.tile([RPC, B], mybir.dt.float32, tag="scr")
                gv = cpool.tile([RPC, k_general], mybir.dt.float32)
                for g in range(k_general):
                    # g = E[p, j[p]] == sum((iota == j) * E) along the row
                    nc.vector.scalar_tensor_tensor(
                        out=scr[:],
                        in0=io[:],
                        scalar=ftab[:, col : col + 1],
                        in1=e_sb[:],
                        op0=mybir.AluOpType.is_equal,
                        op1=mybir.AluOpType.mult,
                        accum_out=gv[:, g : g + 1],
                    )
                    mkg = cpool.tile([RPC, NOUT], mybir.dt.float32, tag="mkg")
                    nc.gpsimd.tensor_scalar(
                        out=mkg[:],
                        in0=io[:, 0:NOUT],
                        scalar1=ftab[:, col + 1 : col + 2],
                        scalar2=gv[:, g : g + 1],
                        op0=mybir.AluOpType.is_equal,
                        op1=mybir.AluOpType.mult,
                    )
                    nc.vector.tensor_tensor(
                        out=y_sb[:], in0=y_sb[:], in1=mkg[:], op=mybir.AluOpType.add
                    )
                    col += 2

            nc.sync.dma_start(out=y_d[:], in_=y_sb[:])
    return nc


def _fast_path(featT, idx):
    """Near-iota idx: returns (nc, in_maps) or None if idx isn't near-iota."""
    iota = np.arange(NOUT, dtype=np.int64)[None, :]
    mism = idx != iota  # (B, NOUT)
    per_row = mism.sum(axis=1)
    kmax = int(per_row.max()) if per_row.size else 0
    if kmax > 24 or mism.sum() > B * 32:
        return None

    # fixup lists per row
    fix_n = [np.nonzero(mism[r])[0] for r in range(B)]  # dest cols
    fix_j = [idx[r, fix_n[r]] for r in range(B)]  # source cols

    # pick up to 2 globally-constant source columns (most frequent j values)
    all_j = np.concatenate(fix_j) if kmax else np.array([], dtype=np.int64)
    const_js = []
    if all_j.size:
        vals, counts = np.unique(all_j, return_counts=True)
        order = np.argsort(-counts)
        for o in order[:2]:
            if counts[o] >= B // 2:  # only worth a slot if most rows use it
                const_js.append(int(vals[o]))
    const_js = tuple(const_js)

    # per-row slot assignment: one slot per const_j (first matching fixup),
    # the rest go to general slots
    n_c = np.full((B, len(const_js)), -1.0, dtype=np.float32)
    gen_lists = [[] for _ in range(B)]
    for r in range(B):
        used = np.zeros(len(fix_n[r]), dtype=bool)
        for ci, cj in enumerate(const_js):
            hits = np.nonzero((fix_j[r] == cj) & ~used)[0]
            if hits.size:
                n_c[r, ci] = float(fix_n[r][hits[0]])
                used[hits[0]] = True
        for t in np.nonzero(~used)[0]:
            gen_lists[r].append((int(fix_j[r][t]), int(fix_n[r][t])))
    k_general = max(len(g) for g in gen_lists) if B else 0

    if k_general > 8:
        return None

    ncols = len(const_js) + 2 * k_general
    ftab = np.zeros((B, ncols), dtype=np.float32)
    ftab[:, : len(const_js)] = n_c
    for r in range(B):
        for s in range(k_general):
            c0 = len(const_js) + 2 * s
            if s < len(gen_lists[r]):
                j, n = gen_lists[r][s]
                ftab[r, c0] = float(j)
                ftab[r, c0 + 1] = float(n)
            else:
                ftab[r, c0] = 0.0
                ftab[r, c0 + 1] = -1.0  # inactive

    zmask = np.ones((B, NOUT), dtype=np.float32)
    rows, cols = np.nonzero(mism)
    zmask[rows, cols] = 0.0

    iota_f = np.arange(B, dtype=np.float32)
    nc = _build_fast(k_general, const_js)
    in_maps = []
    for c in range(NCORES):
        sl = slice(c * RPC, (c + 1) * RPC)
        m = {
            "featT": featT,
            "slabT": np.ascontiguousarray(featT[:, sl]),
            "iota": iota_f,
            "zmask": np.ascontiguousarray(zmask[sl]),
        }
        if ncols:
            m["ftab"] = np.ascontiguousarray(ftab[sl])
        in_maps.append(m)
    return nc, in_maps


# ---------------------------------------------------------------------------
# general fallback: per-element indirect-DMA gather from a DRAM copy of E
# ---------------------------------------------------------------------------
def _build_general():
    nc = bass.Bass()
    featT_d = nc.dram_tensor("featT", [D, B], mybir.dt.float32, kind="ExternalInput")
    slabT_d = nc.dram_tensor("slabT", [D, RPC], mybir.dt.float32, kind="ExternalInput")
    offs_d = nc.dram_tensor("offs", [RPC, NOUT], mybir.dt.int32, kind="ExternalInput")
    y_d = nc.dram_tensor("y", [RPC, NOUT], mybir.dt.float32, kind="ExternalOutput")
    e_dram = nc.dram_tensor("escratch", [RPC * B, 1], mybir.dt.float32)

    with TileContext(nc) as tc:
        with (
            tc.tile_pool(name="const", bufs=1) as cpool,
            tc.tile_pool(name="psum", bufs=2, space="PSUM") as ppool,
        ):
            ft = cpool.tile([D, B], mybir.dt.float32)
            nc.sync.dma_start(out=ft[:], in_=featT_d[:])
            st = cpool.tile([D, RPC], mybir.dt.float32)
            nc.sync.dma_start(out=st[:], in_=slabT_d[:])
            off = cpool.tile([RPC, NOUT], mybir.dt.int32)
            nc.sync.dma_start(out=off[:], in_=offs_d[:])

            e_sb = cpool.tile([RPC, B], mybir.dt.float32)
            for half in range(2):
                ps = ppool.tile([RPC, B // 2], mybir.dt.float32)
                nc.tensor.matmul(
                    ps[:],
                    st[:],
                    ft[:, half * (B // 2) : (half + 1) * (B // 2)],
                    start=True,
                    stop=True,
                )
                nc.scalar.activation(
                    out=e_sb[:, half * (B // 2) : (half + 1) * (B // 2)],
                    in_=ps[:],
                    func=mybir.ActivationFunctionType.Exp,
                    scale=1.0 / T,
                )

            # stage E to DRAM, then per-element gather back by flat offsets
            e_flat = e_dram[:].rearrange("(p n) o -> p (n o)", p=RPC)
            nc.sync.dma_start(out=e_flat, in_=e_sb[:])

            y_sb = cpool.tile([RPC, NOUT], mybir.dt.float32)
            with tc.tile_critical():
                nc.gpsimd.indirect_dma_start(
                    out=y_sb[:],
                    out_offset=None,
                    in_=e_dram[:],
                    in_offset=bass.IndirectOffsetOnAxis(ap=off[:], axis=0),
                )
            nc.sync.dma_start(out=y_d[:], in_=y_sb[:])
    return nc


def _general_path(featT, idx):
    nc = _build_general()
    in_maps = []
    for c in range(NCORES):
        sl = slice(c * RPC, (c + 1) * RPC)
        offs = (
            np.arange(RPC, dtype=np.int64)[:, None] * B + idx[sl].astype(np.int64)
        ).astype(np.int32)
        in_maps.append(
            {
                "featT": featT,
                "slabT": np.ascontiguousarray(featT[:, sl]),
                "offs": np.ascontiguousarray(offs),
            }
        )
    return nc, in_maps


# ---------------------------------------------------------------------------
# entry point
# ---------------------------------------------------------------------------
def kernel(feat, y=None, idx=None):
    feat = np.ascontiguousarray(np.asarray(feat), dtype=np.float32)
    idx = np.asarray(idx)
    assert feat.shape == (B, D), feat.shape
    assert idx.shape == (B, NOUT), idx.shape
    idx_i = idx.astype(np.int64)

    featT = np.ascontiguousarray(feat.T)

    built = _fast_path(featT, idx_i)
    if built is None:
        built = _general_path(featT, idx_i)
    nc, in_maps = built

    res = run_bass_kernel_spmd(
        nc, in_maps, core_ids=list(range(NCORES)), trace=_TRACE
    )
    _last_result["exec_time_ns"] = res.exec_time_ns
    _last_result["mean_exec_time_ns"] = res.mean_exec_time_ns
    _last_result["profile_json"] = res.profile_json

    out = np.concatenate([res.results[c]["y"] for c in range(NCORES)], axis=0)
    return out.astype(np.float32)


# revision 4
# speedup vs baseline: 2.3352x; 2.3352x over previous
"""Trainium2 Bass kernel for nn_BatchAverage (retrieval_knn).

out[b, n] = exp(dot(feat[b], feat[idx[b, n]]) / T)
          = exp(S[b, idx[b, n]] / T)   where S = feat @ feat.T  (Gram matrix)

Strategy (8 NeuronCores, data-parallel over rows):
  - Each core owns a 128-row slab. It computes S_slab = slabT.T @ featT with
    two PE matmuls (contraction over D=128 on partitions), then
    E = exp(S/T) on the ACT engine.
  - The per-row gather E[p, idx[p, :]] is specialized at trace time: for the
    near-iota idx this problem uses (idx[b,n] == n except a few entries per
    row), the output is E[:, :1023] with a handful of per-row fixups, applied
    with per-partition masked ops (tensor_scalar is_equal + tensor_mask_reduce).
  - Arbitrary idx falls back to a per-element indirect-DMA gather from a DRAM
    staging copy of E (correct, slower).
"""

import os
import sys
import types

sys.path.insert(0, "/opt/trn_rl_repo")

import numpy as np

# ---------------------------------------------------------------------------
# optional NTFF tracing shim (exec-time measurement); enabled by KERNEL_TRACE=1
# ---------------------------------------------------------------------------
_TRACE = os.environ.get("KERNEL_TRACE", "0") == "1"
if _TRACE:
    try:
        import antenv

        _hooks_mod = types.ModuleType("antenv.axon_hooks")
        _hook_box = [None]
        _hooks_mod.set_axon_ntff_profile_hook = lambda h: _hook_box.__setitem__(0, h)
        _hooks_mod.get_axon_ntff_profile_hook = lambda: _hook_box[0]
        sys.modules["antenv.axon_hooks"] = _hooks_mod
        antenv.axon_hooks = _hooks_mod
        from trn_agent_boot.trn_boot import _ntff_profile_via_ctypes

        _hooks_mod.set_axon_ntff_profile_hook(
            _ntff_profile_via_ctypes("/opt/axon/libaxon_pjrt.so")
        )
    except Exception:
        _TRACE = False

import concourse.bass as bass
import concourse.mybir as mybir
import concourse.bass_utils as bass_utils
import concourse.tile as tile_mod
from concourse.tile import TileContext
from concourse.vector_clock import ScopedClock
from concourse.bass_utils import run_bass_kernel_spmd

if _TRACE:
    bass_utils.upload_artifacts = lambda tmpdir: "local://" + tmpdir


# ---------------------------------------------------------------------------
# walrus in this container rejects >1 sync-wait on the Tile exit Drain; put
# each wait on its own SP nop before the drain instead.
# ---------------------------------------------------------------------------
def _patched_drain_and_barrier(self, tick_clock, wait_clock):
    carrier = self.nc.sync.nop()
    wait_clock.add_sem_waits(carrier.ins, ScopedClock({None: tick_clock.global_clock}))
    si = carrier.ins.sync_info
    if si is not None and len(si.on_wait) > 1:
        waits = list(si.on_wait)
        si.on_wait = waits[:1]
        for w in waits[1:]:
            extra = self.nc.sync.nop()
            extra.ins.sync_info = mybir.SyncInfo(on_wait=[w], on_update=[])
    self.nc.sync.drain()
    self.nc.all_engine_barrier()
    assert self.sems is not None
    popped = self.nc._tile_sem_poison_stack.pop()
    assert popped is self._sem_poison
    self.nc.clear_and_free_semaphores(list(self.sems.allocated().values()))
    self.nc.all_engine_barrier()


tile_mod.TileContext._drain_and_barrier = _patched_drain_and_barrier


# ---------------------------------------------------------------------------
# same walrus limitation, applied generally: split any instruction carrying
# multiple sync-waits into same-engine NoOps (one wait each) + the original
# instruction with the last wait. Done at the BIR-JSON level on serialization.
# ---------------------------------------------------------------------------
import json as _json

_orig_to_json_bytes = bass.Bass.to_json_bytes


def _split_multi_waits_json(self):
    raw = _orig_to_json_bytes(self)
    j = _json.loads(raw)
    changed = False
    for fn in j.get("functions", []):
        for blk in fn.get("blocks", []):
            out = []
            for ins in blk.get("instructions", []):
                si = ins.get("sync_info")
                waits = (si or {}).get("on_wait") or []
                if len(waits) > 1:
                    changed = True
                    for i, w in enumerate(waits[:-1]):
                        out.append(
                            {
                                "debug": ins.get("debug", 0),
                                "engine": ins["engine"],
                                "ins": [],
                                "name": f"{ins['name']}-ws{i}",
                                "opcode": "NoOp",
                                "outs": [],
                                "sync_info": {"on_wait": [w], "on_update": []},
                            }
                        )
                    si["on_wait"] = [waits[-1]]
                out.append(ins)
            blk["instructions"] = out
    if not changed:
        return raw
    return _json.dumps(j).encode()


bass.Bass.to_json_bytes = _split_multi_waits_json

T = 0.07
B = 1024
D = 128
NCORES = 8
RPC = B // NCORES  # rows per core = 128
NOUT = B - 1  # 1023

_last_result = {}  # test harness reads exec_time_ns etc. from here


# ---------------------------------------------------------------------------
# fast path: near-iota idx -> bulk copy + per-row fixups
# ---------------------------------------------------------------------------
def _build_fast(k_general: int, const_js: tuple[int, ...]):
    """Build the SPMD Bass program.

    k_general: number of general fixup slots (per-row source column, gathered
               with tensor_mask_reduce).
    const_js:  source columns shared by all rows/cores; their fixup slots read
               E[:, j] directly.
    Per-core inputs:
      featT  (D, B)        f32  feat transposed (replicated)
      slabT  (D, RPC)      f32  featT[:, slab] for this core
      iota   (NOUT,)       f32  0..NOUT-1 (replicated, broadcast to partitions)
      zmask  (RPC, NOUT)   f32  1 except 0 at fixup destinations
      ftab   (RPC, NCOLS)  f32  packed per-row fixup table:
             [n_c for each const_j] + [j_g, n_g for each general slot]
             (n == -1 marks an inactive slot)
    Output: y (RPC, NOUT) f32
    """
    ncols = len(const_js) + 2 * k_general
    nc = bass.Bass()
    featT_d = nc.dram_tensor("featT", [D, B], mybir.dt.float32, kind="ExternalInput")
    slabT_d = nc.dram_tensor("slabT", [D, RPC], mybir.dt.float32, kind="ExternalInput")
    iota_d = nc.dram_tensor("iota", [B], mybir.dt.float32, kind="ExternalInput")
    zmask_d = nc.dram_tensor("zmask", [RPC, NOUT], mybir.dt.float32, kind="ExternalInput")
    ftab_d = None
    if ncols:
        ftab_d = nc.dram_tensor(
            "ftab", [RPC, ncols], mybir.dt.float32, kind="ExternalInput"
        )
    y_d = nc.dram_tensor("y", [RPC, NOUT], mybir.dt.float32, kind="ExternalOutput")

    with TileContext(nc) as tc:
        with (
            tc.tile_pool(name="const", bufs=1) as cpool,
            tc.tile_pool(name="psum", bufs=2, space="PSUM") as ppool,
        ):
            ft = cpool.tile([D, B], mybir.dt.float32)
            nc.sync.dma_start(out=ft[:], in_=featT_d[:])
            st = cpool.tile([D, RPC], mybir.dt.float32)
            nc.sync.dma_start(out=st[:], in_=slabT_d[:])
            zm = cpool.tile([RPC, NOUT], mybir.dt.float32)
            nc.scalar.dma_start(out=zm[:], in_=zmask_d[:])
            io = cpool.tile([RPC, B], mybir.dt.float32)
            iota_bcast = bass.AP(
                tensor=iota_d.tensor if hasattr(iota_d, "tensor") else iota_d[:].tensor,
                offset=iota_d[:].offset,
                ap=[[0, RPC]] + list(iota_d[:].ap),
            )
            nc.scalar.dma_start(out=io[:], in_=iota_bcast)
            if ftab_d is not None:
                ftab = cpool.tile([RPC, ncols], mybir.dt.float32)
                nc.scalar.dma_start(out=ftab[:], in_=ftab_d[:])

            # S = slabT.T @ featT, two 512-wide matmuls, then E = exp(S/T)
            e_sb = cpool.tile([RPC, B], mybir.dt.float32)
            for half in range(2):
                ps = ppool.tile([RPC, B // 2], mybir.dt.float32)
                nc.tensor.matmul(
                    ps[:],
                    st[:],
                    ft[:, half * (B // 2) : (half + 1) * (B // 2)],
                    start=True,
                    stop=True,
                )
                nc.scalar.activation(
                    out=e_sb[:, half * (B // 2) : (half + 1) * (B // 2)],
                    in_=ps[:],
                    func=mybir.ActivationFunctionType.Exp,
                    scale=1.0 / T,
                )

            # base: Y = E[:, :NOUT] * zmask
            y_sb = cpool.tile([RPC, NOUT], mybir.dt.float32)
            nc.vector.tensor_tensor(
                out=y_sb[:], in0=e_sb[:, 0:NOUT], in1=zm[:], op=mybir.AluOpType.mult
            )

            col = 0
            # const-source fixups: Y += (iota == n_c) * E[:, j]
            for j in const_js:
                mkg = cpool.tile([RPC, NOUT], mybir.dt.float32, tag="mkg")
                nc.vector.tensor_scalar(
                    out=mkg[:],
                    in0=io[:, 0:NOUT],
                    scalar1=ftab[:, col : col + 1],
                    scalar2=e_sb[:, j : j + 1],
                    op0=mybir.AluOpType.is_equal,
                    op1=mybir.AluOpType.mult,
                )
                nc.vector.tensor_tensor(
                    out=y_sb[:], in0=y_sb[:], in1=mkg[:], op=mybir.AluOpType.add
                )
                col += 1

            # general fixups: g = E[p, j[p]] via masked max-reduce, then
            # Y += (iota == n) * g
            if k_general:
                scr = cpool.tile([RPC, B], mybir.dt.float32, tag="scr")
                gv = cpool.tile([RPC, k_general], mybir.dt.float32)
                for g in range(k_general):
                    # g = E[p, j[p]] == sum((iota == j) * E) along the row
                    nc.vector.scalar_tensor_tensor(
                        out=scr[:],
                        in0=io[:],
                        scalar=ftab[:, col : col + 1],
                        in1=e_sb[:],
                        op0=mybir.AluOpType.is_equal,
                        op1=mybir.AluOpType.mult,
                        accum_out=gv[:, g : g + 1],
                    )
                    mkg = cpool.tile([RPC, NOUT], mybir.dt.float32, tag="mkg")
                    nc.vector.tensor_scalar(
                        out=mkg[:],
                        in0=io[:, 0:NOUT],
                        scalar1=ftab[:, col + 1 : col + 2],
                        scalar2=gv[:, g : g + 1],
                        op0=mybir.AluOpType.is_equal,
                        op1=mybir.AluOpType.mult,
                    )
                    nc.vector.tensor_tensor(
                        out=y_sb[:], in0=y_sb[:], in1=mkg[:], op=mybir.AluOpType.add
                    )
                    col += 2

            nc.sync.dma_start(out=y_d[:], in_=y_sb[:])
    return nc


def _fast_path(featT, idx):
    """Near-iota idx: returns (nc, in_maps) or None if idx isn't near-iota."""
    iota = np.arange(NOUT, dtype=np.int64)[None, :]
    mism = idx != iota  # (B, NOUT)
    per_row = mism.sum(axis=1)
    kmax = int(per_row.max()) if per_row.size else 0
    if kmax > 24 or mism.sum() > B * 32:
        return None

    # fixup lists per row
    fix_n = [np.nonzero(mism[r])[0] for r in range(B)]  # dest cols
    fix_j = [idx[r, fix_n[r]] for r in range(B)]  # source cols

    # pick up to 2 globally-constant source columns (most frequent j values)
    all_j = np.concatenate(fix_j) if kmax else np.array([], dtype=np.int64)
    const_js = []
    if all_j.size:
        vals, counts = np.unique(all_j, return_counts=True)
        order = np.argsort(-counts)
        for o in order[:2]:
            if counts[o] >= B // 2:  # only worth a slot if most rows use it
                const_js.append(int(vals[o]))
    const_js = tuple(const_js)

    # per-row slot assignment: one slot per const_j (first matching fixup),
    # the rest go to general slots
    n_c = np.full((B, len(const_js)), -1.0, dtype=np.float32)
    gen_lists = [[] for _ in range(B)]
    for r in range(B):
        used = np.zeros(len(fix_n[r]), dtype=bool)
        for ci, cj in enumerate(const_js):
            hits = np.nonzero((fix_j[r] == cj) & ~used)[0]
            if hits.size:
                n_c[r, ci] = float(fix_n[r][hits[0]])
                used[hits[0]] = True
        for t in np.nonzero(~used)[0]:
            gen_lists[r].append((int(fix_j[r][t]), int(fix_n[r][t])))
    k_general = max(len(g) for g in gen_lists) if B else 0

    if k_general > 8:
        return None

    ncols = len(const_js) + 2 * k_general
    ftab = np.zeros((B, ncols), dtype=np.float32)
    ftab[:, : len(const_js)] = n_c
    for r in range(B):
        for s in range(k_general):
            c0 = len(const_js) + 2 * s
            if s < len(gen_lists[r]):
                j, n = gen_lists[r][s]
                ftab[r, c0] = float(j)
                ftab[r, c0 + 1] = float(n)
            else:
                ftab[r, c0] = 0.0
                ftab[r, c0 + 1] = -1.0  # inactive

    zmask = np.ones((B, NOUT), dtype=np.float32)
    rows, cols = np.nonzero(mism)
    zmask[rows, cols] = 0.0

    iota_f = np.arange(B, dtype=np.float32)
    nc = _build_fast(k_general, const_js)
    in_maps = []
    for c in range(NCORES):
        sl = slice(c * RPC, (c + 1) * RPC)
        m = {
            "featT": featT,
            "slabT": np.ascontiguousarray(featT[:, sl]),
            "iota": iota_f,
            "zmask": np.ascontiguousarray(zmask[sl]),
        }
        if ncols:
            m["ftab"] = np.ascontiguousarray(ftab[sl])
        in_maps.append(m)
    return nc, in_maps


# ---------------------------------------------------------------------------
# general fallback: per-element indirect-DMA gather from a DRAM copy of E
# ---------------------------------------------------------------------------
def _build_general():
    nc = bass.Bass()
    featT_d = nc.dram_tensor("featT", [D, B], mybir.dt.float32, kind="ExternalInput")
    slabT_d = nc.dram_tensor("slabT", [D, RPC], mybir.dt.float32, kind="ExternalInput")
    offs_d = nc.dram_tensor("offs", [RPC, NOUT], mybir.dt.int32, kind="ExternalInput")
    y_d = nc.dram_tensor("y", [RPC, NOUT], mybir.dt.float32, kind="ExternalOutput")
    e_dram = nc.dram_tensor("escratch", [RPC * B, 1], mybir.dt.float32)

    with TileContext(nc) as tc:
        with (
            tc.tile_pool(name="const", bufs=1) as cpool,
            tc.tile_pool(name="psum", bufs=2, space="PSUM") as ppool,
        ):
            ft = cpool.tile([D, B], mybir.dt.float32)
            nc.sync.dma_start(out=ft[:], in_=featT_d[:])
            st = cpool.tile([D, RPC], mybir.dt.float32)
            nc.sync.dma_start(out=st[:], in_=slabT_d[:])
            off = cpool.tile([RPC, NOUT], mybir.dt.int32)
            nc.sync.dma_start(out=off[:], in_=offs_d[:])

            e_sb = cpool.tile([RPC, B], mybir.dt.float32)
            for half in range(2):
                ps = ppool.tile([RPC, B // 2], mybir.dt.float32)
                nc.tensor.matmul(
                    ps[:],
                    st[:],
                    ft[:, half * (B // 2) : (half + 1) * (B // 2)],
                    start=True,
                    stop=True,
                )
                nc.scalar.activation(
                    out=e_sb[:, half * (B // 2) : (half + 1) * (B // 2)],
                    in_=ps[:],
                    func=mybir.ActivationFunctionType.Exp,
                    scale=1.0 / T,
                )

            # stage E to DRAM, then per-element gather back by flat offsets
            e_flat = e_dram[:].rearrange("(p n) o -> p (n o)", p=RPC)
            nc.sync.dma_start(out=e_flat, in_=e_sb[:])

            y_sb = cpool.tile([RPC, NOUT], mybir.dt.float32)
            with tc.tile_critical():
                nc.gpsimd.indirect_dma_start(
                    out=y_sb[:],
                    out_offset=None,
                    in_=e_dram[:],
                    in_offset=bass.IndirectOffsetOnAxis(ap=off[:], axis=0),
                )
            nc.sync.dma_start(out=y_d[:], in_=y_sb[:])
    return nc


def _general_path(featT, idx):
    nc = _build_general()
    in_maps = []
    for c in range(NCORES):
        sl = slice(c * RPC, (c + 1) * RPC)
        offs = (
            np.arange(RPC, dtype=np.int64)[:, None] * B + idx[sl].astype(np.int64)
        ).astype(np.int32)
        in_maps.append(
            {
                "featT": featT,
                "slabT": np.ascontiguousarray(featT[:, sl]),
                "offs": np.ascontiguousarray(offs),
            }
        )
    return nc, in_maps


# ---------------------------------------------------------------------------
# entry point
# ---------------------------------------------------------------------------
def kernel(feat, y=None, idx=None):
    feat = np.ascontiguousarray(np.asarray(feat), dtype=np.float32)
    idx = np.asarray(idx)
    assert feat.shape == (B, D), feat.shape
    assert idx.shape == (B, NOUT), idx.shape
    idx_i = idx.astype(np.int64)

    featT = np.ascontiguousarray(feat.T)

    built = _fast_path(featT, idx_i)
    if built is None:
        built = _general_path(featT, idx_i)
    nc, in_maps = built

    res = run_bass_kernel_spmd(
        nc, in_maps, core_ids=list(range(NCORES)), trace=_TRACE
    )
    _last_result["exec_time_ns"] = res.exec_time_ns
    _last_result["mean_exec_time_ns"] = res.mean_exec_time_ns
    _last_result["profile_json"] = res.profile_json

    out = np.concatenate([res.results[c]["y"] for c in range(NCORES)], axis=0)
    return out.astype(np.float32)


# revision 8
# speedup vs baseline: 2.4638x; 1.0551x over previous
"""Trainium2 Bass kernel for nn_BatchAverage (retrieval_knn).

out[b, n] = exp(dot(feat[b], feat[idx[b, n]]) / T)
          = exp(S[b, idx[b, n]] / T)   where S = feat @ feat.T  (Gram matrix)

Strategy (8 NeuronCores, data-parallel over rows):
  - Each core owns a 128-row slab. It computes S_slab = slabT.T @ featT with
    two PE matmuls (contraction over D=128 on partitions), then
    E = exp(S/T) on the ACT engine.
  - The per-row gather E[p, idx[p, :]] is specialized at trace time: for the
    near-iota idx this problem uses (idx[b,n] == n except a few entries per
    row), the output is E[:, :1023] with a handful of per-row fixups:
      * slot D  (dest column constant): per-row source gather via
        sum((iota == j[p]) * E) on DVE, then a single-column overwrite.
      * T1 slots (source column constant): one fused
        Y = (mask * E[:, j]) + Y scalar_tensor_tensor per slot, with a
        host-precomputed 0/1 mask.
      * T3 slots (both per-row): iota-gather + fused masked add.
  - Arbitrary idx falls back to a per-element indirect-DMA gather from a DRAM
    staging copy of E (correct, slower).
"""

import os
import sys
import types

sys.path.insert(0, "/opt/trn_rl_repo")

import numpy as np

# ---------------------------------------------------------------------------
# optional NTFF tracing shim (exec-time measurement); enabled by KERNEL_TRACE=1
# ---------------------------------------------------------------------------
_TRACE = os.environ.get("KERNEL_TRACE", "0") == "1"
if _TRACE:
    try:
        import antenv

        _hooks_mod = types.ModuleType("antenv.axon_hooks")
        _hook_box = [None]
        _hooks_mod.set_axon_ntff_profile_hook = lambda h: _hook_box.__setitem__(0, h)
        _hooks_mod.get_axon_ntff_profile_hook = lambda: _hook_box[0]
        sys.modules["antenv.axon_hooks"] = _hooks_mod
        antenv.axon_hooks = _hooks_mod
        from trn_agent_boot.trn_boot import _ntff_profile_via_ctypes

        _hooks_mod.set_axon_ntff_profile_hook(
            _ntff_profile_via_ctypes("/opt/axon/libaxon_pjrt.so")
        )
    except Exception:
        _TRACE = False

import concourse.bass as bass
import concourse.mybir as mybir
import concourse.bass_utils as bass_utils
import concourse.tile as tile_mod
from concourse.tile import TileContext
from concourse.vector_clock import ScopedClock
from concourse.bass_utils import run_bass_kernel_spmd

if _TRACE:
    bass_utils.upload_artifacts = lambda tmpdir: "local://" + tmpdir


# ---------------------------------------------------------------------------
# walrus in this container rejects >1 sync-wait per instruction; Tile's exit
# Drain carries several. Spread them across SP nops.
# ---------------------------------------------------------------------------
def _patched_drain_and_barrier(self, tick_clock, wait_clock):
    carrier = self.nc.sync.nop()
    wait_clock.add_sem_waits(carrier.ins, ScopedClock({None: tick_clock.global_clock}))
    si = carrier.ins.sync_info
    if si is not None and len(si.on_wait) > 1:
        waits = list(si.on_wait)
        si.on_wait = waits[:1]
        for w in waits[1:]:
            extra = self.nc.sync.nop()
            extra.ins.sync_info = mybir.SyncInfo(on_wait=[w], on_update=[])
    self.nc.sync.drain()
    self.nc.all_engine_barrier()
    assert self.sems is not None
    popped = self.nc._tile_sem_poison_stack.pop()
    assert popped is self._sem_poison
    self.nc.clear_and_free_semaphores(list(self.sems.allocated().values()))
    self.nc.all_engine_barrier()


tile_mod.TileContext._drain_and_barrier = _patched_drain_and_barrier


# same limitation applied generally at BIR-JSON serialization time: any
# instruction with multiple sync-waits gets same-engine NoOps carrying the
# extra waits inserted in front of it.
import json as _json

_orig_to_json_bytes = bass.Bass.to_json_bytes


def _split_multi_waits_json(self):
    raw = _orig_to_json_bytes(self)
    j = _json.loads(raw)
    changed = False
    for fn in j.get("functions", []):
        for blk in fn.get("blocks", []):
            out = []
            for ins in blk.get("instructions", []):
                si = ins.get("sync_info")
                waits = (si or {}).get("on_wait") or []
                if len(waits) > 1:
                    changed = True
                    for i, w in enumerate(waits[:-1]):
                        out.append(
                            {
                                "debug": ins.get("debug", 0),
                                "engine": ins["engine"],
                                "ins": [],
                                "name": f"{ins['name']}-ws{i}",
                                "opcode": "NoOp",
                                "outs": [],
                                "sync_info": {"on_wait": [w], "on_update": []},
                            }
                        )
                    si["on_wait"] = [waits[-1]]
                out.append(ins)
            blk["instructions"] = out
    if not changed:
        return raw
    return _json.dumps(j).encode()


bass.Bass.to_json_bytes = _split_multi_waits_json

T = 0.07
B = 1024
D = 128
NCORES = 8
RPC = B // NCORES  # rows per core = 128
NOUT = B - 1  # 1023

_last_result = {}  # test harness reads exec_time_ns etc. from here

F32 = mybir.dt.float32
ALU = mybir.AluOpType

from contextlib import ExitStack


# ---------------------------------------------------------------------------
# fast path device program (raw bass, manual semaphores)
# ---------------------------------------------------------------------------
def _build_fast_raw(plan):
    """plan: dict with keys
    has_fixups: bool
    slot_d: None | {'n': int}            (gather j column -> jtab col 0)
    t1_slots: [{'j': int}]               (mask inputs mask0, mask1, ...)
    t3_slots: count                      (jtab cols after slot_d; masks after t1)
    """
    has_fix = plan["has_fixups"]
    slot_d = plan["slot_d"]
    t1 = plan["t1_slots"]
    t3 = plan["t3_slots"]
    n_gather = (1 if slot_d else 0) + t3
    n_masks = len(t1) + t3

    nc = bass.Bass()
    featT_d = nc.dram_tensor("featT", [D, B], F32, kind="ExternalInput")
    slabT_d = nc.dram_tensor("slabT", [D, RPC], F32, kind="ExternalInput")
    y_d = nc.dram_tensor("y", [RPC, NOUT], F32, kind="ExternalOutput")
    zmask_d = iota_d = jtab_d = None
    mask_ds = []
    if has_fix:
        zmask_d = nc.dram_tensor("zmask", [RPC, NOUT], F32, kind="ExternalInput")
        if n_gather:
            iota_d = nc.dram_tensor("iota", [B], F32, kind="ExternalInput")
            jtab_d = nc.dram_tensor("jtab", [RPC, n_gather], F32, kind="ExternalInput")
        for m in range(n_masks):
            mask_ds.append(
                nc.dram_tensor(f"mask{m}", [RPC, NOUT], F32, kind="ExternalInput")
            )

    n_scalar_dmas = (1 if has_fix else 0) + (2 if n_gather else 0) + n_masks

    with (
        nc.sbuf_tensor([D, B], F32) as ft,
        nc.sbuf_tensor([D, RPC], F32) as st,
        nc.sbuf_tensor([RPC, B], F32) as e_sb,
        nc.sbuf_tensor([RPC, NOUT], F32) as y_sb,
        nc.sbuf_tensor([RPC, NOUT], F32) as zm,
        nc.sbuf_tensor([RPC, B], F32) as io,
        nc.sbuf_tensor([RPC, max(n_gather, 1)], F32) as jt,
        nc.sbuf_tensor([RPC, max(n_gather, 1)], F32) as gv,
        nc.sbuf_tensor([RPC, B], F32) as scr,
        nc.sbuf_tensor([RPC, 1], F32) as warm,
        ExitStack() as mstack,
        nc.semaphore("ds") as ds,  # sync-queue DMA completions
        nc.semaphore("dsc") as dsc,  # scalar-queue DMA completions
        nc.semaphore("pe") as pe,
        nc.semaphore("act") as act,
        nc.semaphore("dve") as dve,
        nc.semaphore("dout") as dout,
        nc.psum_tensor([RPC, B // 2], F32) as ps0,
        nc.psum_tensor([RPC, B // 2], F32) as ps1,
        nc.Block(no_gpsimd_drain=True) as block,
    ):
        masks = [
            mstack.enter_context(nc.sbuf_tensor(f"mask{i}_sb", [RPC, NOUT], F32))
            for i in range(n_masks)
        ]

        @block.sync
        def _(sync):
            sync.dma_start(out=st[:], in_=slabT_d[:]).then_inc(ds, 16)
            sync.dma_start(out=ft[:, 0 : B // 2], in_=featT_d[:, 0 : B // 2]).then_inc(
                ds, 16
            )
            sync.dma_start(out=ft[:, B // 2 : B], in_=featT_d[:, B // 2 : B]).then_inc(
                ds, 16
            )
            sync.wait_ge(dve, 1)
            sync.dma_start(out=y_d[:], in_=y_sb[:]).then_inc(dout, 16)
            sync.wait_ge(dout, 16)

        @block.scalar
        def _(scalar):
            # warmup: trigger the EXP table load while DMAs are in flight
            scalar.activation(
                out=warm[:],
                in_=nc.const_aps.aps[(F32, 0.0)],
                func=mybir.ActivationFunctionType.Exp,
                scale=1.0,
            )
            if has_fix:
                if n_gather:
                    iota_bcast = bass.AP(
                        tensor=iota_d[:].tensor,
                        offset=iota_d[:].offset,
                        ap=[[0, RPC]] + list(iota_d[:].ap),
                    )
                    scalar.dma_start(out=io[:], in_=iota_bcast).then_inc(dsc, 16)
                    scalar.dma_start(out=jt[:], in_=jtab_d[:]).then_inc(dsc, 16)
                scalar.dma_start(out=zm[:], in_=zmask_d[:]).then_inc(dsc, 16)
                for m in range(n_masks):
                    scalar.dma_start(out=masks[m][:], in_=mask_ds[m][:]).then_inc(
                        dsc, 16
                    )
            scalar.wait_ge(pe, 1)
            scalar.activation(
                out=e_sb[:, 0 : B // 2],
                in_=ps0[:],
                func=mybir.ActivationFunctionType.Exp,
                scale=1.0 / T,
            )
            scalar.drain().then_inc(act, 1)
            scalar.wait_ge(pe, 2)
            scalar.activation(
                out=e_sb[:, B // 2 : B],
                in_=ps1[:],
                func=mybir.ActivationFunctionType.Exp,
                scale=1.0 / T,
            )
            scalar.drain().then_inc(act, 1)

        @block.tensor
        def _(tensor):
            tensor.wait_ge(ds, 32)
            nc.tensor.matmul(
                ps0[:], st[:], ft[:, 0 : B // 2], start=True, stop=True
            ).then_inc(pe, 1)
            tensor.wait_ge(ds, 48)
            nc.tensor.matmul(
                ps1[:], st[:], ft[:, B // 2 : B], start=True, stop=True
            ).then_inc(pe, 1)

        @block.vector
        def _(vector):
            if n_scalar_dmas:
                vector.wait_ge(dsc, 16 * n_scalar_dmas)
            vector.wait_ge(act, 1)
            if has_fix:
                vector.tensor_tensor(
                    out=y_sb[:, 0 : B // 2],
                    in0=e_sb[:, 0 : B // 2],
                    in1=zm[:, 0 : B // 2],
                    op=ALU.mult,
                )
                vector.wait_ge(act, 2)
                vector.tensor_tensor(
                    out=y_sb[:, B // 2 : NOUT],
                    in0=e_sb[:, B // 2 : NOUT],
                    in1=zm[:, B // 2 : NOUT],
                    op=ALU.mult,
                )
            else:
                vector.tensor_copy(out=y_sb[:, 0 : B // 2], in_=e_sb[:, 0 : B // 2])
                vector.wait_ge(act, 2)
                vector.tensor_copy(
                    out=y_sb[:, B // 2 : NOUT], in_=e_sb[:, B // 2 : NOUT]
                )

            gcol = 0
            if slot_d is not None:
                # gv = E[p, jt[p,0]] via sum((iota == j) * E)
                vector.scalar_tensor_tensor(
                    out=scr[:],
                    in0=io[:],
                    scalar=jt[:, 0:1],
                    in1=e_sb[:],
                    op0=ALU.is_equal,
                    op1=ALU.mult,
                    accum_out=gv[:, 0:1],
                )
                # accum_out is only readable by the next DVE op after a drain
                vector.drain()
                nd = slot_d["n"]
                vector.tensor_copy(out=y_sb[:, nd : nd + 1], in_=gv[:, 0:1])
                gcol = 1
            for si_ in range(len(t1)):
                jj = t1[si_]["j"]
                vector.scalar_tensor_tensor(
                    out=y_sb[:],
                    in0=masks[si_][:],
                    scalar=e_sb[:, jj : jj + 1],
                    in1=y_sb[:],
                    op0=ALU.mult,
                    op1=ALU.add,
                )
            for g in range(t3):
                vector.scalar_tensor_tensor(
                    out=scr[:],
                    in0=io[:],
                    scalar=jt[:, gcol : gcol + 1],
                    in1=e_sb[:],
                    op0=ALU.is_equal,
                    op1=ALU.mult,
                    accum_out=gv[:, gcol : gcol + 1],
                )
                vector.drain()
                vector.scalar_tensor_tensor(
                    out=y_sb[:],
                    in0=masks[len(t1) + g][:],
                    scalar=gv[:, gcol : gcol + 1],
                    in1=y_sb[:],
                    op0=ALU.mult,
                    op1=ALU.add,
                )
                gcol += 1
            vector.drain().then_inc(dve, 1)

    return nc


def _fast_path(featT, idx):
    """Near-iota idx: returns (nc, in_maps) or None."""
    iota = np.arange(NOUT, dtype=np.int64)[None, :]
    mism = idx != iota  # (B, NOUT)
    total = int(mism.sum())
    if total == 0:
        nc = _build_fast_raw(
            {"has_fixups": False, "slot_d": None, "t1_slots": [], "t3_slots": 0}
        )
        in_maps = []
        for c in range(NCORES):
            sl = slice(c * RPC, (c + 1) * RPC)
            in_maps.append(
                {"featT": featT, "slabT": np.ascontiguousarray(featT[:, sl])}
            )
        return nc, in_maps

    per_row = mism.sum(axis=1)
    if int(per_row.max()) > 8 or total > B * 16:
        return None

    fix = [
        [(int(idx[r, n]), int(n)) for n in np.nonzero(mism[r])[0]] for r in range(B)
    ]  # per row: list of (j, n)

    # slot D: most common destination column
    all_n = np.array([n for f in fix for _, n in f], dtype=np.int64)
    n_vals, n_counts = np.unique(all_n, return_counts=True)
    slot_d = None
    jtab_cols = []
    if n_counts.max() >= B // 2:
        mode_n = int(n_vals[np.argmax(n_counts)])
        jd = np.empty(B, dtype=np.float32)
        for r in range(B):
            hit = [t for t in fix[r] if t[1] == mode_n]
            if hit:
                jd[r] = float(hit[0][0])
                fix[r].remove(hit[0])
            else:
                # harmless self-write: value at that position is the base value
                jd[r] = float(idx[r, mode_n])
        slot_d = {"n": mode_n}
        jtab_cols.append(jd)

    # T1 slots: constant source columns among remaining fixups
    t1_slots = []
    mask_arrays = []
    while True:
        rem_j = np.array([t[0] for f in fix for t in f], dtype=np.int64)
        if rem_j.size == 0:
            break
        j_vals, j_counts = np.unique(rem_j, return_counts=True)
        top = int(np.argmax(j_counts))
        if j_counts[top] < B // 2 or len(t1_slots) >= 4:
            break
        jstar = int(j_vals[top])
        m = np.zeros((B, NOUT), dtype=np.float32)
        for r in range(B):
            hit = [t for t in fix[r] if t[0] == jstar]
            if hit:
                m[r, hit[0][1]] = 1.0
                fix[r].remove(hit[0])
        t1_slots.append({"j": jstar})
        mask_arrays.append(m)

    # T3 slots: whatever is left
    t3 = max(len(f) for f in fix)
    if t3 > 4:
        return None
    for s in range(t3):
        jg = np.zeros(B, dtype=np.float32)
        m = np.zeros((B, NOUT), dtype=np.float32)
        for r in range(B):
            if s < len(fix[r]):
                j, n = fix[r][s]
                jg[r] = float(j)
                m[r, n] = 1.0
        jtab_cols.append(jg)
        mask_arrays.append(m)

    zmask = np.ones((B, NOUT), dtype=np.float32)
    rows, cols = np.nonzero(mism)
    zmask[rows, cols] = 0.0

    n_gather = len(jtab_cols)
    jtab = (
        np.stack(jtab_cols, axis=1).astype(np.float32)
        if n_gather
        else np.zeros((B, 1), np.float32)
    )
    iota_f = np.arange(B, dtype=np.float32)

    plan = {
        "has_fixups": True,
        "slot_d": slot_d,
        "t1_slots": t1_slots,
        "t3_slots": t3,
    }
    nc = _build_fast_raw(plan)
    in_maps = []
    for c in range(NCORES):
        sl = slice(c * RPC, (c + 1) * RPC)
        m = {
            "featT": featT,
            "slabT": np.ascontiguousarray(featT[:, sl]),
            "zmask": np.ascontiguousarray(zmask[sl]),
        }
        if n_gather:
            m["iota"] = iota_f
            m["jtab"] = np.ascontiguousarray(jtab[sl])
        for mi, ma in enumerate(mask_arrays):
            m[f"mask{mi}"] = np.ascontiguousarray(ma[sl])
        in_maps.append(m)
    return nc, in_maps


# ---------------------------------------------------------------------------
# general fallback: per-element indirect-DMA gather from a DRAM copy of E
# ---------------------------------------------------------------------------
def _build_general():
    nc = bass.Bass()
    featT_d = nc.dram_tensor("featT", [D, B], F32, kind="ExternalInput")
    slabT_d = nc.dram_tensor("slabT", [D, RPC], F32, kind="ExternalInput")
    offs_d = nc.dram_tensor("offs", [RPC, NOUT], mybir.dt.int32, kind="ExternalInput")
    y_d = nc.dram_tensor("y", [RPC, NOUT], F32, kind="ExternalOutput")
    e_dram = nc.dram_tensor("escratch", [RPC * B, 1], F32)

    with (
        nc.sbuf_tensor([D, B], F32) as ft,
        nc.sbuf_tensor([D, RPC], F32) as st,
        nc.sbuf_tensor([RPC, NOUT], mybir.dt.int32) as off,
        nc.sbuf_tensor([RPC, B], F32) as e_sb,
        nc.sbuf_tensor([RPC, NOUT], F32) as y_sb,
        nc.semaphore("ds") as ds,
        nc.semaphore("pe") as pe,
        nc.semaphore("act") as act,
        nc.semaphore("de") as de,  # E staged to DRAM
        nc.semaphore("dg") as dg,  # gather done
        nc.semaphore("dout") as dout,
        nc.psum_tensor([RPC, B // 2], F32) as ps0,
        nc.psum_tensor([RPC, B // 2], F32) as ps1,
        nc.Block() as block,
    ):
        @block.sync
        def _(sync):
            sync.dma_start(out=st[:], in_=slabT_d[:]).then_inc(ds, 16)
            sync.dma_start(out=ft[:], in_=featT_d[:]).then_inc(ds, 16)
            sync.dma_start(out=off[:], in_=offs_d[:]).then_inc(ds, 16)
            sync.wait_ge(act, 2)
            sync.dma_start(
                out=e_dram[:].rearrange("(p n) o -> p (n o)", p=RPC), in_=e_sb[:]
            ).then_inc(de, 16)
            sync.wait_ge(dg, 16)
            sync.dma_start(out=y_d[:], in_=y_sb[:]).then_inc(dout, 16)
            sync.wait_ge(dout, 16)

        @block.scalar
        def _(scalar):
            scalar.wait_ge(pe, 1)
            scalar.activation(
                out=e_sb[:, 0 : B // 2],
                in_=ps0[:],
                func=mybir.ActivationFunctionType.Exp,
                scale=1.0 / T,
            )
            scalar.drain().then_inc(act, 1)
            scalar.wait_ge(pe, 2)
            scalar.activation(
                out=e_sb[:, B // 2 : B],
                in_=ps1[:],
                func=mybir.ActivationFunctionType.Exp,
                scale=1.0 / T,
            )
            scalar.drain().then_inc(act, 1)

        @block.tensor
        def _(tensor):
            tensor.wait_ge(ds, 32)
            nc.tensor.matmul(
                ps0[:], st[:], ft[:, 0 : B // 2], start=True, stop=True
            ).then_inc(pe, 1)
            nc.tensor.matmul(
                ps1[:], st[:], ft[:, B // 2 : B], start=True, stop=True
            ).then_inc(pe, 1)

        @block.gpsimd
        def _(gpsimd):
            gpsimd.wait_ge(de, 16)
            gpsimd.wait_ge(ds, 48)
            gpsimd.indirect_dma_start(
                out=y_sb[:],
                out_offset=None,
                in_=e_dram[:],
                in_offset=bass.IndirectOffsetOnAxis(ap=off[:], axis=0),
            ).then_inc(dg, 16)

    return nc


def _general_path(featT, idx):
    nc = _build_general()
    in_maps = []
    for c in range(NCORES):
        sl = slice(c * RPC, (c + 1) * RPC)
        offs = (
            np.arange(RPC, dtype=np.int64)[:, None] * B + idx[sl].astype(np.int64)
        ).astype(np.int32)
        in_maps.append(
            {
                "featT": featT,
                "slabT": np.ascontiguousarray(featT[:, sl]),
                "offs": np.ascontiguousarray(offs),
            }
        )
    return nc, in_maps


# ---------------------------------------------------------------------------
# entry point
# ---------------------------------------------------------------------------
def kernel(feat, y=None, idx=None):
    feat = np.ascontiguousarray(np.asarray(feat), dtype=np.float32)
    idx = np.asarray(idx)
    assert feat.shape == (B, D), feat.shape
    assert idx.shape == (B, NOUT), idx.shape
    idx_i = idx.astype(np.int64)

    featT = np.ascontiguousarray(feat.T)

    built = _fast_path(featT, idx_i)
    if built is None:
        built = _general_path(featT, idx_i)
    nc, in_maps = built

    res = run_bass_kernel_spmd(nc, in_maps, core_ids=list(range(NCORES)), trace=_TRACE)
    _last_result["exec_time_ns"] = res.exec_time_ns
    _last_result["mean_exec_time_ns"] = res.mean_exec_time_ns
    _last_result["profile_json"] = res.profile_json

    out = np.concatenate([res.results[c]["y"] for c in range(NCORES)], axis=0)
    return out.astype(np.float32)


# revision 9
# speedup vs baseline: 2.4972x; 1.0136x over previous
"""Trainium2 Bass kernel for nn_BatchAverage (retrieval_knn).

out[b, n] = exp(dot(feat[b], feat[idx[b, n]]) / T)
          = exp(S[b, idx[b, n]] / T)   where S = feat @ feat.T  (Gram matrix)

Strategy (8 NeuronCores, data-parallel over rows):
  - Each core owns a 128-row slab. It computes S_slab = slabT.T @ featT with
    two PE matmuls (contraction over D=128 on partitions), then
    E = exp(S/T) on the ACT engine.
  - The per-row gather E[p, idx[p, :]] is specialized at trace time: for the
    near-iota idx this problem uses (idx[b,n] == n except a few entries per
    row), the output is E[:, :1023] with a handful of per-row fixups:
      * slot D  (dest column constant): per-row source gather via
        sum((iota == j[p]) * E) on DVE, then a single-column overwrite.
      * T1 slots (source column constant): one fused
        Y = (mask * E[:, j]) + Y scalar_tensor_tensor per slot, with a
        host-precomputed 0/1 mask.
      * T3 slots (both per-row): iota-gather + fused masked add.
  - Arbitrary idx falls back to a per-element indirect-DMA gather from a DRAM
    staging copy of E (correct, slower).
"""

import os
import sys
import types

sys.path.insert(0, "/opt/trn_rl_repo")

import numpy as np

# ---------------------------------------------------------------------------
# optional NTFF tracing shim (exec-time measurement); enabled by KERNEL_TRACE=1
# ---------------------------------------------------------------------------
_TRACE = os.environ.get("KERNEL_TRACE", "0") == "1"
if _TRACE:
    try:
        import antenv

        _hooks_mod = types.ModuleType("antenv.axon_hooks")
        _hook_box = [None]
        _hooks_mod.set_axon_ntff_profile_hook = lambda h: _hook_box.__setitem__(0, h)
        _hooks_mod.get_axon_ntff_profile_hook = lambda: _hook_box[0]
        sys.modules["antenv.axon_hooks"] = _hooks_mod
        antenv.axon_hooks = _hooks_mod
        from trn_agent_boot.trn_boot import _ntff_profile_via_ctypes

        _hooks_mod.set_axon_ntff_profile_hook(
            _ntff_profile_via_ctypes("/opt/axon/libaxon_pjrt.so")
        )
    except Exception:
        _TRACE = False

import concourse.bass as bass
import concourse.mybir as mybir
import concourse.bass_utils as bass_utils
import concourse.tile as tile_mod
from concourse.tile import TileContext
from concourse.vector_clock import ScopedClock
from concourse.bass_utils import run_bass_kernel_spmd

if _TRACE:
    bass_utils.upload_artifacts = lambda tmpdir: "local://" + tmpdir


# ---------------------------------------------------------------------------
# walrus in this container rejects >1 sync-wait per instruction; Tile's exit
# Drain carries several. Spread them across SP nops.
# ---------------------------------------------------------------------------
def _patched_drain_and_barrier(self, tick_clock, wait_clock):
    carrier = self.nc.sync.nop()
    wait_clock.add_sem_waits(carrier.ins, ScopedClock({None: tick_clock.global_clock}))
    si = carrier.ins.sync_info
    if si is not None and len(si.on_wait) > 1:
        waits = list(si.on_wait)
        si.on_wait = waits[:1]
        for w in waits[1:]:
            extra = self.nc.sync.nop()
            extra.ins.sync_info = mybir.SyncInfo(on_wait=[w], on_update=[])
    self.nc.sync.drain()
    self.nc.all_engine_barrier()
    assert self.sems is not None
    popped = self.nc._tile_sem_poison_stack.pop()
    assert popped is self._sem_poison
    self.nc.clear_and_free_semaphores(list(self.sems.allocated().values()))
    self.nc.all_engine_barrier()


tile_mod.TileContext._drain_and_barrier = _patched_drain_and_barrier


# same limitation applied generally at BIR-JSON serialization time: any
# instruction with multiple sync-waits gets same-engine NoOps carrying the
# extra waits inserted in front of it.
import json as _json

_orig_to_json_bytes = bass.Bass.to_json_bytes


def _split_multi_waits_json(self):
    raw = _orig_to_json_bytes(self)
    j = _json.loads(raw)
    changed = False
    for fn in j.get("functions", []):
        for blk in fn.get("blocks", []):
            out = []
            for ins in blk.get("instructions", []):
                si = ins.get("sync_info")
                waits = (si or {}).get("on_wait") or []
                if len(waits) > 1:
                    changed = True
                    for i, w in enumerate(waits[:-1]):
                        out.append(
                            {
                                "debug": ins.get("debug", 0),
                                "engine": ins["engine"],
                                "ins": [],
                                "name": f"{ins['name']}-ws{i}",
                                "opcode": "NoOp",
                                "outs": [],
                                "sync_info": {"on_wait": [w], "on_update": []},
                            }
                        )
                    si["on_wait"] = [waits[-1]]
                out.append(ins)
            blk["instructions"] = out
    if not changed:
        return raw
    return _json.dumps(j).encode()


bass.Bass.to_json_bytes = _split_multi_waits_json

T = 0.07
B = 1024
D = 128
NCORES = 8
RPC = B // NCORES  # rows per core = 128
NOUT = B - 1  # 1023

_last_result = {}  # test harness reads exec_time_ns etc. from here

F32 = mybir.dt.float32
ALU = mybir.AluOpType

from contextlib import ExitStack


# ---------------------------------------------------------------------------
# fast path device program (raw bass, manual semaphores)
# ---------------------------------------------------------------------------
def _build_fast_raw(plan):
    """plan: dict with keys
    has_fixups: bool
    slot_d: None | {'n': int}            (gather j column -> jtab col 0)
    t1_slots: [{'j': int}]               (mask inputs mask0, mask1, ...)
    t3_slots: count                      (jtab cols after slot_d; masks after t1)
    """
    has_fix = plan["has_fixups"]
    slot_d = plan["slot_d"]
    t1 = plan["t1_slots"]
    t3 = plan["t3_slots"]
    n_gather = (1 if slot_d else 0) + t3
    n_masks = len(t1) + t3

    nc = bass.Bass()
    featT_d = nc.dram_tensor("featT", [D, B], F32, kind="ExternalInput")
    slabT_d = nc.dram_tensor("slabT", [D, RPC], F32, kind="ExternalInput")
    y_d = nc.dram_tensor("y", [RPC, NOUT], F32, kind="ExternalOutput")
    zmask_d = iota_d = jtab_d = None
    mask_ds = []
    if has_fix:
        zmask_d = nc.dram_tensor("zmask", [RPC, NOUT], F32, kind="ExternalInput")
        if n_gather:
            iota_d = nc.dram_tensor("iota", [B], F32, kind="ExternalInput")
            jtab_d = nc.dram_tensor("jtab", [RPC, n_gather], F32, kind="ExternalInput")
        for m in range(n_masks):
            mask_ds.append(
                nc.dram_tensor(f"mask{m}", [RPC, NOUT], F32, kind="ExternalInput")
            )

    n_scalar_dmas = (1 if has_fix else 0) + (2 if n_gather else 0) + n_masks

    with (
        nc.sbuf_tensor([D, B], F32) as ft,
        nc.sbuf_tensor([D, RPC], F32) as st,
        nc.sbuf_tensor([RPC, B], F32) as e_sb,
        nc.sbuf_tensor([RPC, NOUT], F32) as y_sb,
        nc.sbuf_tensor([RPC, NOUT], F32) as zm,
        nc.sbuf_tensor([RPC, B], F32) as io,
        nc.sbuf_tensor([RPC, max(n_gather, 1)], F32) as jt,
        nc.sbuf_tensor([RPC, max(n_gather, 1)], F32) as gv,
        nc.sbuf_tensor([RPC, B], F32) as scr,
        nc.sbuf_tensor([RPC, 1], F32) as warm,
        ExitStack() as mstack,
        nc.semaphore("ds") as ds,  # sync-queue DMA completions
        nc.semaphore("dsc") as dsc,  # scalar-queue DMA completions
        nc.semaphore("pe") as pe,
        nc.semaphore("act") as act,
        nc.semaphore("dve") as dve,
        nc.semaphore("dout") as dout,
        nc.psum_tensor([RPC, B // 2], F32) as ps0,
        nc.psum_tensor([RPC, B // 2], F32) as ps1,
        nc.Block(no_gpsimd_drain=True) as block,
    ):
        masks = [
            mstack.enter_context(nc.sbuf_tensor(f"mask{i}_sb", [RPC, NOUT], F32))
            for i in range(n_masks)
        ]

        @block.sync
        def _(sync):
            sync.dma_start(out=st[:], in_=slabT_d[:]).then_inc(ds, 16)
            sync.dma_start(out=ft[:, 0 : B // 2], in_=featT_d[:, 0 : B // 2]).then_inc(
                ds, 16
            )
            sync.dma_start(out=ft[:, B // 2 : B], in_=featT_d[:, B // 2 : B]).then_inc(
                ds, 16
            )
            sync.wait_ge(dve, 1)
            sync.dma_start(out=y_d[:], in_=y_sb[:]).then_inc(dout, 16)
            sync.wait_ge(dout, 16)

        @block.scalar
        def _(scalar):
            # warmup: trigger the EXP table load while DMAs are in flight
            scalar.activation(
                out=warm[:],
                in_=nc.const_aps.aps[(F32, 0.0)],
                func=mybir.ActivationFunctionType.Exp,
                scale=1.0,
            )
            if has_fix:
                if n_gather:
                    iota_bcast = bass.AP(
                        tensor=iota_d[:].tensor,
                        offset=iota_d[:].offset,
                        ap=[[0, RPC]] + list(iota_d[:].ap),
                    )
                    scalar.dma_start(out=io[:], in_=iota_bcast).then_inc(dsc, 16)
                    scalar.dma_start(out=jt[:], in_=jtab_d[:]).then_inc(dsc, 16)
                scalar.dma_start(out=zm[:], in_=zmask_d[:]).then_inc(dsc, 16)
                for m in range(n_masks):
                    scalar.dma_start(out=masks[m][:], in_=mask_ds[m][:]).then_inc(
                        dsc, 16
                    )
            scalar.wait_ge(pe, 1)
            scalar.activation(
                out=e_sb[:, 0 : B // 2],
                in_=ps0[:],
                func=mybir.ActivationFunctionType.Exp,
                scale=1.0 / T,
            )
            scalar.drain().then_inc(act, 1)
            scalar.wait_ge(pe, 2)
            scalar.activation(
                out=e_sb[:, B // 2 : B],
                in_=ps1[:],
                func=mybir.ActivationFunctionType.Exp,
                scale=1.0 / T,
            )
            scalar.drain().then_inc(act, 1)

        @block.tensor
        def _(tensor):
            tensor.wait_ge(ds, 32)
            nc.tensor.matmul(ps0[:], st[:], ft[:, 0 : B // 2], start=True, stop=True)
            tensor.drain().then_inc(pe, 1)
            tensor.wait_ge(ds, 48)
            nc.tensor.matmul(ps1[:], st[:], ft[:, B // 2 : B], start=True, stop=True)
            tensor.drain().then_inc(pe, 1)

        @block.vector
        def _(vector):
            if n_scalar_dmas:
                vector.wait_ge(dsc, 16 * n_scalar_dmas)
            vector.wait_ge(act, 1)
            if has_fix:
                vector.tensor_tensor(
                    out=y_sb[:, 0 : B // 2],
                    in0=e_sb[:, 0 : B // 2],
                    in1=zm[:, 0 : B // 2],
                    op=ALU.mult,
                )
                vector.wait_ge(act, 2)
                vector.tensor_tensor(
                    out=y_sb[:, B // 2 : NOUT],
                    in0=e_sb[:, B // 2 : NOUT],
                    in1=zm[:, B // 2 : NOUT],
                    op=ALU.mult,
                )
            else:
                vector.tensor_copy(out=y_sb[:, 0 : B // 2], in_=e_sb[:, 0 : B // 2])
                vector.wait_ge(act, 2)
                vector.tensor_copy(
                    out=y_sb[:, B // 2 : NOUT], in_=e_sb[:, B // 2 : NOUT]
                )

            gcol = 0
            if slot_d is not None:
                # gv = E[p, jt[p,0]] via sum((iota == j) * E)
                vector.scalar_tensor_tensor(
                    out=scr[:],
                    in0=io[:],
                    scalar=jt[:, 0:1],
                    in1=e_sb[:],
                    op0=ALU.is_equal,
                    op1=ALU.mult,
                    accum_out=gv[:, 0:1],
                )
                # accum_out is only readable by the next DVE op after a drain
                vector.drain()
                nd = slot_d["n"]
                vector.tensor_copy(out=y_sb[:, nd : nd + 1], in_=gv[:, 0:1])
                gcol = 1
            for si_ in range(len(t1)):
                jj = t1[si_]["j"]
                vector.scalar_tensor_tensor(
                    out=y_sb[:],
                    in0=masks[si_][:],
                    scalar=e_sb[:, jj : jj + 1],
                    in1=y_sb[:],
                    op0=ALU.mult,
                    op1=ALU.add,
                )
            for g in range(t3):
                vector.scalar_tensor_tensor(
                    out=scr[:],
                    in0=io[:],
                    scalar=jt[:, gcol : gcol + 1],
                    in1=e_sb[:],
                    op0=ALU.is_equal,
                    op1=ALU.mult,
                    accum_out=gv[:, gcol : gcol + 1],
                )
                vector.drain()
                vector.scalar_tensor_tensor(
                    out=y_sb[:],
                    in0=masks[len(t1) + g][:],
                    scalar=gv[:, gcol : gcol + 1],
                    in1=y_sb[:],
                    op0=ALU.mult,
                    op1=ALU.add,
                )
                gcol += 1
            vector.drain().then_inc(dve, 1)

    return nc


def _fast_path(featT, idx):
    """Near-iota idx: returns (nc, in_maps) or None."""
    iota = np.arange(NOUT, dtype=np.int64)[None, :]
    mism = idx != iota  # (B, NOUT)
    total = int(mism.sum())
    if total == 0:
        nc = _build_fast_raw(
            {"has_fixups": False, "slot_d": None, "t1_slots": [], "t3_slots": 0}
        )
        in_maps = []
        for c in range(NCORES):
            sl = slice(c * RPC, (c + 1) * RPC)
            in_maps.append(
                {"featT": featT, "slabT": np.ascontiguousarray(featT[:, sl])}
            )
        return nc, in_maps

    per_row = mism.sum(axis=1)
    if int(per_row.max()) > 8 or total > B * 16:
        return None

    fix = [
        [(int(idx[r, n]), int(n)) for n in np.nonzero(mism[r])[0]] for r in range(B)
    ]  # per row: list of (j, n)

    # slot D: most common destination column
    all_n = np.array([n for f in fix for _, n in f], dtype=np.int64)
    n_vals, n_counts = np.unique(all_n, return_counts=True)
    slot_d = None
    jtab_cols = []
    if n_counts.max() >= B // 2:
        mode_n = int(n_vals[np.argmax(n_counts)])
        jd = np.empty(B, dtype=np.float32)
        for r in range(B):
            hit = [t for t in fix[r] if t[1] == mode_n]
            if hit:
                jd[r] = float(hit[0][0])
                fix[r].remove(hit[0])
            else:
                # harmless self-write: value at that position is the base value
                jd[r] = float(idx[r, mode_n])
        slot_d = {"n": mode_n}
        jtab_cols.append(jd)

    # T1 slots: constant source columns among remaining fixups
    t1_slots = []
    mask_arrays = []
    while True:
        rem_j = np.array([t[0] for f in fix for t in f], dtype=np.int64)
        if rem_j.size == 0:
            break
        j_vals, j_counts = np.unique(rem_j, return_counts=True)
        top = int(np.argmax(j_counts))
        if j_counts[top] < B // 2 or len(t1_slots) >= 4:
            break
        jstar = int(j_vals[top])
        m = np.zeros((B, NOUT), dtype=np.float32)
        for r in range(B):
            hit = [t for t in fix[r] if t[0] == jstar]
            if hit:
                m[r, hit[0][1]] = 1.0
                fix[r].remove(hit[0])
        t1_slots.append({"j": jstar})
        mask_arrays.append(m)

    # T3 slots: whatever is left
    t3 = max(len(f) for f in fix)
    if t3 > 4:
        return None
    for s in range(t3):
        jg = np.zeros(B, dtype=np.float32)
        m = np.zeros((B, NOUT), dtype=np.float32)
        for r in range(B):
            if s < len(fix[r]):
                j, n = fix[r][s]
                jg[r] = float(j)
                m[r, n] = 1.0
        jtab_cols.append(jg)
        mask_arrays.append(m)

    zmask = np.ones((B, NOUT), dtype=np.float32)
    rows, cols = np.nonzero(mism)
    zmask[rows, cols] = 0.0

    n_gather = len(jtab_cols)
    jtab = (
        np.stack(jtab_cols, axis=1).astype(np.float32)
        if n_gather
        else np.zeros((B, 1), np.float32)
    )
    iota_f = np.arange(B, dtype=np.float32)

    plan = {
        "has_fixups": True,
        "slot_d": slot_d,
        "t1_slots": t1_slots,
        "t3_slots": t3,
    }
    nc = _build_fast_raw(plan)
    in_maps = []
    for c in range(NCORES):
        sl = slice(c * RPC, (c + 1) * RPC)
        m = {
            "featT": featT,
            "slabT": np.ascontiguousarray(featT[:, sl]),
            "zmask": np.ascontiguousarray(zmask[sl]),
        }
        if n_gather:
            m["iota"] = iota_f
            m["jtab"] = np.ascontiguousarray(jtab[sl])
        for mi, ma in enumerate(mask_arrays):
            m[f"mask{mi}"] = np.ascontiguousarray(ma[sl])
        in_maps.append(m)
    return nc, in_maps


# ---------------------------------------------------------------------------
# general fallback: per-element indirect-DMA gather from a DRAM copy of E
# ---------------------------------------------------------------------------
def _build_general():
    nc = bass.Bass()
    featT_d = nc.dram_tensor("featT", [D, B], F32, kind="ExternalInput")
    slabT_d = nc.dram_tensor("slabT", [D, RPC], F32, kind="ExternalInput")
    offs_d = nc.dram_tensor("offs", [RPC, NOUT], mybir.dt.int32, kind="ExternalInput")
    y_d = nc.dram_tensor("y", [RPC, NOUT], F32, kind="ExternalOutput")
    e_dram = nc.dram_tensor("escratch", [RPC * B, 1], F32)

    with (
        nc.sbuf_tensor([D, B], F32) as ft,
        nc.sbuf_tensor([D, RPC], F32) as st,
        nc.sbuf_tensor([RPC, NOUT], mybir.dt.int32) as off,
        nc.sbuf_tensor([RPC, B], F32) as e_sb,
        nc.sbuf_tensor([RPC, NOUT], F32) as y_sb,
        nc.semaphore("ds") as ds,
        nc.semaphore("pe") as pe,
        nc.semaphore("act") as act,
        nc.semaphore("de") as de,  # E staged to DRAM
        nc.semaphore("dg") as dg,  # gather done
        nc.semaphore("dout") as dout,
        nc.psum_tensor([RPC, B // 2], F32) as ps0,
        nc.psum_tensor([RPC, B // 2], F32) as ps1,
        nc.Block() as block,
    ):
        @block.sync
        def _(sync):
            sync.dma_start(out=st[:], in_=slabT_d[:]).then_inc(ds, 16)
            sync.dma_start(out=ft[:], in_=featT_d[:]).then_inc(ds, 16)
            sync.dma_start(out=off[:], in_=offs_d[:]).then_inc(ds, 16)
            sync.wait_ge(act, 2)
            sync.dma_start(
                out=e_dram[:].rearrange("(p n) o -> p (n o)", p=RPC), in_=e_sb[:]
            ).then_inc(de, 16)
            sync.wait_ge(dg, 16)
            sync.dma_start(out=y_d[:], in_=y_sb[:]).then_inc(dout, 16)
            sync.wait_ge(dout, 16)

        @block.scalar
        def _(scalar):
            scalar.wait_ge(pe, 1)
            scalar.activation(
                out=e_sb[:, 0 : B // 2],
                in_=ps0[:],
                func=mybir.ActivationFunctionType.Exp,
                scale=1.0 / T,
            )
            scalar.drain().then_inc(act, 1)
            scalar.wait_ge(pe, 2)
            scalar.activation(
                out=e_sb[:, B // 2 : B],
                in_=ps1[:],
                func=mybir.ActivationFunctionType.Exp,
                scale=1.0 / T,
            )
            scalar.drain().then_inc(act, 1)

        @block.tensor
        def _(tensor):
            tensor.wait_ge(ds, 32)
            nc.tensor.matmul(ps0[:], st[:], ft[:, 0 : B // 2], start=True, stop=True)
            tensor.drain().then_inc(pe, 1)
            nc.tensor.matmul(ps1[:], st[:], ft[:, B // 2 : B], start=True, stop=True)
            tensor.drain().then_inc(pe, 1)

        @block.gpsimd
        def _(gpsimd):
            gpsimd.wait_ge(de, 16)
            gpsimd.wait_ge(ds, 48)
            gpsimd.indirect_dma_start(
                out=y_sb[:],
                out_offset=None,
                in_=e_dram[:],
                in_offset=bass.IndirectOffsetOnAxis(ap=off[:], axis=0),
            ).then_inc(dg, 16)

    return nc


def _general_path(featT, idx):
    nc = _build_general()
    in_maps = []
    for c in range(NCORES):
        sl = slice(c * RPC, (c + 1) * RPC)
        offs = (
            np.arange(RPC, dtype=np.int64)[:, None] * B + idx[sl].astype(np.int64)
        ).astype(np.int32)
        in_maps.append(
            {
                "featT": featT,
                "slabT": np.ascontiguousarray(featT[:, sl]),
                "offs": np.ascontiguousarray(offs),
            }
        )
    return nc, in_maps


# ---------------------------------------------------------------------------
# entry point
# ---------------------------------------------------------------------------
def kernel(feat, y=None, idx=None):
    feat = np.ascontiguousarray(np.asarray(feat), dtype=np.float32)
    idx = np.asarray(idx)
    assert feat.shape == (B, D), feat.shape
    assert idx.shape == (B, NOUT), idx.shape
    idx_i = idx.astype(np.int64)

    featT = np.ascontiguousarray(feat.T)

    built = _fast_path(featT, idx_i)
    if built is None:
        built = _general_path(featT, idx_i)
    nc, in_maps = built

    res = run_bass_kernel_spmd(nc, in_maps, core_ids=list(range(NCORES)), trace=_TRACE)
    _last_result["exec_time_ns"] = res.exec_time_ns
    _last_result["mean_exec_time_ns"] = res.mean_exec_time_ns
    _last_result["profile_json"] = res.profile_json

    out = np.concatenate([res.results[c]["y"] for c in range(NCORES)], axis=0)
    return out.astype(np.float32)
